# revision 1
# baseline (speedup 1.0000x reference)
"""Deformable Conv2d (DeformConv2dPack) Trainium2 Bass kernel — v4 (bf16).

Same algorithm as v3.x (bf16 sampling path, row-pair interleaved scratch,
one batched gather per slab, packed DVE bilinear, row-sum folded into the
deform matmul). v4 change: the gather-destination pool is allocated BEFORE
the scoped startup pools, so pool-release barriers cannot gate the first
gathers; slab-0 index math + wrap run ahead and gather 0 launches as soon
as the scratch is written.
"""

import os
import sys

sys.path.insert(0, "/opt/trn_rl_repo")

import numpy as np
import ml_dtypes

import concourse.bacc as bacc
import concourse.bass as bass
import concourse.mybir as mybir
from concourse import masks
from concourse.bass_utils import run_bass_kernel_spmd
from concourse.tile import TileContext

F32 = mybir.dt.float32
BF16 = mybir.dt.bfloat16
I32 = mybir.dt.int32
I16 = mybir.dt.int16

B, CIN, COUT, H, W = 4, 64, 64, 128, 128
K2 = 9
ROWS = 96
SCOLS = 160
NUNITS = ROWS * SCOLS
UNIT = 128
MAGIC = 12582912.0
CLAMP = 12.0
NSLABS = 8
SLAB = 8
ALU = mybir.AluOpType
ACTF = mybir.ActivationFunctionType
BF16NP = ml_dtypes.bfloat16
NPOOL = int(os.environ.get("AXK_NPOOL", "4"))
POOL_TAPS = set(range(1, 1 + 2 * NPOOL, 2)) if NPOOL else set()


def _emit(tc, xs2, woffA, woffB, boffx4, wdx2, bdef, yout):
    nc = tc.nc

    with (
        tc.tile_pool(name="const", bufs=1) as cpool,
        tc.tile_pool(name="gat", bufs=3) as gpool,
        tc.tile_pool(name="dram", bufs=1, space="DRAM") as dpool,
    ):
        scratch = dpool.tile([NUNITS, UNIT], BF16)
        scr_h = scratch[:].tensor

        zero_sb = cpool.tile([48, 2048], BF16)
        nc.vector.memset(zero_sb[:], 0.0)
        for half in range(2):
            r0 = 48 * half
            nc.sync.dma_start(
                out=bass.AP(scr_h, r0 * SCOLS * UNIT,
                            [[SCOLS * UNIT, 48], [1, 16 * UNIT]]),
                in_=zero_sb[:],
            )
            nc.sync.dma_start(
                out=bass.AP(scr_h, (r0 * SCOLS + 144) * UNIT,
                            [[SCOLS * UNIT, 48], [1, 16 * UNIT]]),
                in_=zero_sb[:],
            )
        nc.sync.dma_start(
            out=bass.AP(scr_h, (95 * SCOLS + 16) * UNIT, [[2048, 8], [1, 2048]]),
            in_=zero_sb[0:8, :],
        )

        ident = cpool.tile([128, 128], BF16)
        masks.make_identity(nc, ident[:])
        woffA_sb = cpool.tile([128, 3, 18], BF16)
        nc.sync.dma_start(out=woffA_sb[:], in_=woffA[:])
        woffB_sb = cpool.tile([64, 3, 18], BF16)
        nc.sync.dma_start(out=woffB_sb[:], in_=woffB[:])
        boffx4_sb = cpool.tile([32, 4, 18], BF16)
        nc.sync.dma_start(out=boffx4_sb[:], in_=boffx4[:])
        wdx2_sb = cpool.tile([128, 9, 64], BF16)
        nc.sync.dma_start(out=wdx2_sb[:], in_=wdx2[:])
        bdef_sb = cpool.tile([64, 1], F32)
        nc.sync.dma_start(out=bdef_sb[:], in_=bdef[:])
        ones_sb = cpool.tile([32, 128], BF16)
        nc.vector.memset(ones_sb[:], 0.0)
        nc.vector.memset(ones_sb[0:1, :], 1.0)

        wq2 = cpool.tile([128, 9, 64, 2, 1, 2], BF16)
        wrapped0 = cpool.tile([128, 1, 9, 8, 8], I16)
        wrappedR = cpool.tile([128, 7, 9, 8, 8], I16)
        off_sb = cpool.tile([128, 64, 18], F32)

        with (
            tc.tile_pool(name="xs", bufs=1) as xpool,
            tc.tile_pool(name="stg", bufs=3) as stpool,
            tc.tile_pool(name="wtmp", bufs=1) as wpool,
            tc.tile_pool(name="selp", bufs=1) as selpool,
            tc.tile_pool(name="ps_prep", bufs=2, space="PSUM") as pprep,
            tc.tile_pool(name="ps_conv", bufs=2, space="PSUM") as pconv,
            tc.tile_pool(name="ps_wrap", bufs=2, space="PSUM") as pwrap,
        ):
            xs = xpool.tile([128, 96, 130], BF16)
            nc.sync.dma_start(out=xs[0:64, :, :], in_=xs2[0:64, :, :])
            nc.sync.dma_start(out=xs[64:128, :, :], in_=xs2[64:128, :, :])
            pps_tiles = []

            def trans_block(b):
                u0 = 8 * b
                pps = pprep.tile([128, 8, 64], BF16, tag="prep_ps")
                for j in range(8):
                    nc.tensor.transpose(
                        pps[:, j, :], xs[0:64, u0 + j, 1:129], ident[0:64, 0:64]
                    )
                pps_tiles.append(pps)

            def ilv_block(b):
                u0 = 8 * b
                nu = 8 if b < 11 else 7
                stgi = stpool.tile([128, 8, 64, 2], BF16, tag="stgi")
                nc.vector.tensor_copy(
                    out=stgi[:, 0:nu, :, 0], in_=pps_tiles[b][:, 0:nu, :]
                )
                nc.vector.tensor_copy(
                    out=stgi[:, 0 : nu - 1, :, 1], in_=pps_tiles[b][:, 1:nu, :]
                )
                src = pps_tiles[b + 1][:, 0, :] if b < 11 else pps_tiles[b][:, 7, :]
                nc.vector.tensor_copy(out=stgi[:, nu - 1, :, 1], in_=src)
                nc.sync.dma_start(
                    out=bass.AP(
                        scr_h,
                        (u0 * SCOLS + 16) * UNIT,
                        [[UNIT, 128], [SCOLS * UNIT, nu], [1, UNIT]],
                    ),
                    in_=stgi[:, 0:nu, :, :],
                )

            def conv_block(g4):
                cps = pconv.tile([128, 4, 32], F32, tag="conv_ps")
                for j in range(4):
                    g = 4 * g4 + j
                    for kw in range(3):
                        nc.tensor.matmul(
                            cps[:, j, 0:18],
                            lhsT=xs[:, g + 15, kw : kw + 128],
                            rhs=woffA_sb[:, kw, :],
                            start=(kw == 0),
                            stop=False,
                        )
                    for kw in range(3):
                        nc.tensor.matmul(
                            cps[:, j, 0:18],
                            lhsT=xs[0:64, g + 17, kw : kw + 128],
                            rhs=woffB_sb[:, kw, :],
                            start=False,
                            stop=False,
                        )
                    nc.tensor.matmul(
                        cps[:, j, 0:18],
                        lhsT=ones_sb[:],
                        rhs=boffx4_sb[:, 0, :],
                        start=False,
                        stop=True,
                    )
                nc.scalar.copy(
                    out=off_sb[:, 4 * g4 : 4 * g4 + 4, :], in_=cps[:, :, 0:18]
                )

            dcy = wpool.tile([128, 64, 9], F32, tag="dcy")
            iyf = wpool.tile([128, 64, 9], F32, tag="iyf")
            dcx = wpool.tile([128, 64, 9], F32, tag="dcx")
            ixf = wpool.tile([128, 64, 9], F32, tag="ixf")
            idxg = wpool.tile([128, 64, 9], F32, tag="idxg")
            idxf2 = wpool.tile([128, 8, 9, 8], F32, tag="idxf2")
            basei = wpool.tile([128, 64, 3, 3], I32, tag="basei")
            nc.gpsimd.iota(
                out=basei[:],
                pattern=[[SCOLS, 64], [SCOLS, 3], [1, 3]],
                base=15 * SCOLS + 15,
                channel_multiplier=1,
            )
            basef = wpool.tile([128, 64, 9], F32, tag="basef")
            nc.vector.tensor_copy(
                out=basef[:], in_=basei[:].rearrange("p g a b -> p g (a b)")
            )
            selbase = wpool.tile([128, 128], I32, tag="selbase")
            nc.gpsimd.iota(
                out=selbase[:],
                pattern=[[0, 8], [-1, 16]],
                base=0,
                channel_multiplier=1,
            )
            sels = []
            for p1 in range(8):
                sel = selpool.tile([128, 128], F32, tag=f"sel{p1}")
                nc.vector.tensor_scalar(
                    out=sel[:], in0=selbase[:], scalar1=float(p1 * 16),
                    scalar2=None, op0=ALU.is_equal,
                )
                sels.append(sel)

            off4 = off_sb[:].rearrange("p g (k two) -> p g k two", two=2)

            def idx_math(g0, g1):
                sl = slice(g0, g1)
                for d, dc, fl in ((off4[:, sl, :, 0], dcy, iyf),
                                  (off4[:, sl, :, 1], dcx, ixf)):
                    nc.vector.tensor_scalar(
                        out=dc[:, sl], in0=d, scalar1=CLAMP, scalar2=-CLAMP,
                        op0=ALU.min, op1=ALU.max,
                    )
                    nc.vector.tensor_scalar(
                        out=fl[:, sl], in0=dc[:, sl], scalar1=0.5, scalar2=MAGIC,
                        op0=ALU.subtract, op1=ALU.add,
                    )
                    nc.vector.tensor_scalar(
                        out=fl[:, sl], in0=fl[:, sl], scalar1=MAGIC, scalar2=None,
                        op0=ALU.subtract,
                    )
                nc.vector.scalar_tensor_tensor(
                    out=idxg[:, sl], in0=iyf[:, sl], scalar=float(SCOLS),
                    in1=ixf[:, sl], op0=ALU.mult, op1=ALU.add,
                )
                nc.vector.tensor_tensor(
                    out=idxg[:, sl], in0=idxg[:, sl], in1=basef[:, sl], op=ALU.add
                )
                s0, s1_ = g0 // 8, g1 // 8
                nc.vector.tensor_copy(
                    out=idxf2[:, s0:s1_].rearrange("p s k g -> p s g k"),
                    in_=idxg[:, sl].rearrange("p (s g) k -> p s g k", g=8),
                )

            idxv = idxf2[:].rearrange("p s k g -> p (s k g)")

            for b in range(12):
                trans_block(b)
            conv_block(0)
            conv_block(1)
            for b in range(6):
                ilv_block(b)
            idx_math(0, 8)
            for p1 in range(8):
                wpsA = pwrap.tile([128, 576], F32, tag="wrapA")
                nc.tensor.matmul(
                    wpsA[:, 0:72], lhsT=sels[p1][:], rhs=idxv[:, 0:72],
                    start=True, stop=True,
                )
                nc.scalar.copy(
                    out=wrapped0[:, 0, :, :, p1],
                    in_=wpsA[:, 0:72].rearrange("p (k g) -> p k g", k=9),
                )
            for b in range(6, 12):
                ilv_block(b)
            for g4 in range(2, 16):
                conv_block(g4)
            idx_math(8, 64)
            for p1 in range(8):
                wpsA = pwrap.tile([128, 576], F32, tag="wrapA")
                nc.tensor.matmul(
                    wpsA[:, 72:288], lhsT=sels[p1][:], rhs=idxv[:, 72:288],
                    start=True, stop=True,
                )
                nc.tensor.matmul(
                    wpsA[:, 288:512], lhsT=sels[p1][:], rhs=idxv[:, 288:512],
                    start=True, stop=True,
                )
                nc.tensor.matmul(
                    wpsA[:, 512:576], lhsT=sels[p1][:], rhs=idxv[:, 512:576],
                    start=True, stop=True,
                )
                if p1 % 2 == 0:
                    nc.scalar.copy(
                        out=wrappedR[:, 0:6, :, :, p1],
                        in_=wpsA[:, 72:504].rearrange(
                            "p (s k g) -> p s k g", s=6, k=9
                        ),
                    )
                    nc.scalar.copy(
                        out=wrappedR[:, 6, :, :, p1],
                        in_=wpsA[:, 504:576].rearrange("p (k g) -> p k g", k=9),
                    )
                else:
                    nc.vector.tensor_copy(
                        out=wrappedR[:, 0:6, :, :, p1],
                        in_=wpsA[:, 72:504].rearrange(
                            "p (s k g) -> p s k g", s=6, k=9
                        ),
                    )
                    nc.vector.tensor_copy(
                        out=wrappedR[:, 6, :, :, p1],
                        in_=wpsA[:, 504:576].rearrange("p (k g) -> p k g", k=9),
                    )

            fy = wpool.tile([128, 64, 9], F32, tag="fy")
            nc.vector.tensor_tensor(out=fy[:], in0=dcy[:], in1=iyf[:], op=ALU.subtract)
            fx = wpool.tile([128, 64, 9], F32, tag="fx")
            nc.vector.tensor_tensor(out=fx[:], in0=dcx[:], in1=ixf[:], op=ALU.subtract)
            fy0 = wpool.tile([128, 64, 9], F32, tag="fy0")
            nc.scalar.activation(out=fy0[:], in_=fy[:], func=ACTF.Identity, bias=1.0, scale=-1.0)
            fx0 = wpool.tile([128, 64, 9], F32, tag="fx0")
            nc.scalar.activation(out=fx0[:], in_=fx[:], func=ACTF.Identity, bias=1.0, scale=-1.0)
            for c, wxc in ((0, fx0), (1, fx)):
                for r, wyr in ((0, fy0), (1, fy)):
                    nc.vector.tensor_tensor(
                        out=wq2[:, :, :, c, 0, r],
                        in0=wxc[:].rearrange("p g k -> p k g"),
                        in1=wyr[:].rearrange("p g k -> p k g"),
                        op=ALU.mult,
                    )

        def gather_slab(s):
            gat = gpool.tile([128, 9, 8, 256], BF16, tag="gat")
            win = min((8 * s + 38) * SCOLS, NUNITS - 1)
            nc.gpsimd.dma_gather(
                out_ap=gat[:].rearrange("p k g e -> p (k g) e"),
                in_ap=bass.AP(scr_h, 0, [[UNIT, win], [1, 256]]),
                idxs_ap=(wrapped0[:, 0] if s == 0
                         else wrappedR[:, s - 1]).rearrange(
                    "p k g q -> p (k g q)"),
                num_idxs=9216,
                num_idxs_reg=9216,
                elem_size=256,
                elem_step=UNIT,
                single_packet=False,
            )
            return gat

        with (
            tc.tile_pool(name="prod", bufs=3) as prpool,
            tc.tile_pool(name="smp", bufs=2) as smpool,
            tc.tile_pool(name="trs", bufs=2) as trpool,
            tc.tile_pool(name="outs", bufs=2) as outpool,
            tc.tile_pool(name="ps_tr", bufs=2, space="PSUM") as ptr,
            tc.tile_pool(name="ps_out", bufs=2, space="PSUM") as pout,
        ):
            for s in range(NSLABS):
                gat = gather_slab(s)
                sampled2 = smpool.tile([128, 8, 9, 64, 2], BF16)
                for k in range(9):
                    prod = prpool.tile([128, 16, 64, 2], BF16, tag="prod")
                    gk = gat[:, k].rearrange(
                        "p g (c two r) -> p (g c) two r", c=2, r=2
                    )
                    wk = wq2[:, k, 8 * s : 8 * s + 8].rearrange(
                        "p g c d r -> p (g c) d r"
                    ).broadcast_to([128, 16, 64, 2])
                    nc.vector.tensor_tensor(out=prod[:], in0=gk, in1=wk, op=ALU.mult)
                    pv = prod[:].rearrange("p (g c) ch r -> p g c ch r", c=2)
                    eng = nc.gpsimd if k in POOL_TAPS else nc.vector
                    eng.tensor_tensor(
                        out=sampled2[:, :, k, :, :],
                        in0=pv[:, :, 0],
                        in1=pv[:, :, 1],
                        op=ALU.add,
                    )

                ostg = outpool.tile([64, 8, 128], BF16)
                for g2 in range(8):
                    trp = ptr.tile([128, 9, 128], BF16, tag="trp")
                    for k in range(9):
                        nc.tensor.transpose(
                            trp[:, k, :],
                            sampled2[:, g2, k, :, :].rearrange("p c r -> p (c r)"),
                            ident[:],
                        )
                    trs = trpool.tile([128, 9, 128], BF16)
                    nc.scalar.copy(out=trs[:], in_=trp[:])
                    ops = pout.tile([64, 128], F32, tag="out_ps")
                    for k in range(9):
                        nc.tensor.matmul(
                            ops[:],
                            lhsT=wdx2_sb[:, k, :],
                            rhs=trs[:, k, :],
                            start=(k == 0),
                            stop=(k == 8),
                        )
                    nc.scalar.activation(
                        out=ostg[:, g2, :],
                        in_=ops[:],
                        func=ACTF.Identity,
                        bias=bdef_sb[:],
                        scale=1.0,
                    )
                nc.sync.dma_start(out=yout[:, 8 * s : 8 * s + 8, :], in_=ostg[:])


_CACHE = {}


def _build():
    key = "nc"
    if key in _CACHE:
        return _CACHE[key]
    nc = bacc.Bacc("TRN2", target_bir_lowering=False, debug=False)
    xs2 = nc.dram_tensor("xs2", [128, ROWS, 130], BF16, kind="ExternalInput")
    woffA = nc.dram_tensor("woffA", [128, 3, 18], BF16, kind="ExternalInput")
    woffB = nc.dram_tensor("woffB", [64, 3, 18], BF16, kind="ExternalInput")
    boffx4 = nc.dram_tensor("boffx4", [32, 4, 18], BF16, kind="ExternalInput")
    wdx2 = nc.dram_tensor("wdx2", [128, 9, 64], BF16, kind="ExternalInput")
    bdef = nc.dram_tensor("bdef", [64, 1], F32, kind="ExternalInput")
    yout = nc.dram_tensor("yout", [64, 64, 128], BF16, kind="ExternalOutput")
    with TileContext(nc) as tc:
        _emit(tc, xs2.ap(), woffA.ap(), woffB.ap(), boffx4.ap(), wdx2.ap(),
              bdef.ap(), yout.ap())
    nc.compile()
    _CACHE[key] = nc
    return nc


def make_in_maps(x, w_offset, b_offset, w_deform, b_deform):
    x = np.asarray(x, dtype=np.float32)
    wo = np.asarray(w_offset, np.float32).transpose(1, 2, 3, 0)
    woffA_r = np.zeros((128, 3, 18), np.float32)
    woffA_r[0:64] = wo[:, 0]
    woffA_r[64:128] = wo[:, 1]
    woffA_r = woffA_r.astype(BF16NP)
    woffB_r = np.ascontiguousarray(wo[:, 2]).astype(BF16NP)
    boffx4_r = np.zeros((32, 4, 18), np.float32)
    boffx4_r[0, :, :] = np.asarray(b_offset, np.float32)[None, :]
    boffx4_r = boffx4_r.astype(BF16NP)
    wdr = np.asarray(w_deform, np.float32).transpose(2, 3, 1, 0).reshape(9, 64, 64)
    wdx2_r = np.zeros((128, 9, 64), np.float32)
    wdx2_r[0::2] = wdr.transpose(1, 0, 2)
    wdx2_r[1::2] = wdr.transpose(1, 0, 2)
    wdx2_r = wdx2_r.astype(BF16NP)
    bdef_r = np.asarray(b_deform, np.float32).reshape(64, 1)

    in_maps = []
    for core in range(8):
        b = core // 2
        h0 = (core % 2) * 64
        xrow = np.zeros((64, ROWS + 1, 130), np.float32)
        lo = h0 - 16
        hi = h0 + 81
        src_lo = max(lo, 0)
        src_hi = min(hi, H)
        xrow[:, src_lo - lo : src_hi - lo, 1:129] = x[b, :, src_lo:src_hi, :]
        xs2_r = np.zeros((128, ROWS, 130), np.float32)
        xs2_r[0:64] = xrow[:, 0:96]
        xs2_r[64:128] = xrow[:, 1:97]
        in_maps.append(
            {
                "xs2": np.ascontiguousarray(xs2_r.astype(BF16NP)),
                "woffA": woffA_r,
                "woffB": woffB_r,
                "boffx4": boffx4_r,
                "wdx2": wdx2_r,
                "bdef": bdef_r,
            }
        )
    return in_maps


def kernel(x, w_offset, b_offset, w_deform, b_deform, _trace=False):
    nc = _build()
    in_maps = make_in_maps(x, w_offset, b_offset, w_deform, b_deform)
    res = run_bass_kernel_spmd(nc, in_maps, core_ids=list(range(8)), trace=_trace)
    out = np.zeros((B, COUT, H, W), np.float32)
    for core in range(8):
        b = core // 2
        h0 = (core % 2) * 64
        out[b, :, h0 : h0 + 64, :] = res.results[core]["yout"].astype(np.float32)
    if _trace:
        kernel.last_results = res
    return out

